# revision 13
# baseline (speedup 1.0000x reference)
"""Correlation-volume kernel for Trainium2 (8 NeuronCores, SPMD).

Problem: inputs (B=4, N=2, C=128, H=128, W=128) fp32.
  q = floor(inputs * 1e10) / 1e10  (straight-through quantization, fp32)
  src = q[:, 0], tgt = q[:, 1]
  out[b, dy*21+dx, h, w] = mean_c src[b,c,h,w] * tgt[b,c,h+dy-10,w+dx-10]
  (zero padding outside), out shape (4, 441, 128, 128) fp32.

Strategy:
  - Shard batch(4) x H-half(2) across 8 cores, data parallel, no collectives.
  - Host precomputes q, casts to bf16, pre-blocks src into 128-pixel
    stationary tiles (16 h x 8 w), zero-pads tgt; one packed input per core.
  - Device: per block, 2 bf16 matmuls (K=C=128, M=128 pixels,
    N=18 tgt rows x 28 tgt cols = 504) -> PSUM fp32; DVE/ACT copies cast to
    bf16 into a per-group staging tile.
  - Zoned dump: partition group h_l (8 partitions) only needs Gram t-rows
    h_l..h_l+20, so the 128 partitions split into 4 zones of 32 (h_l 4z..4z+3)
    and each zone dumps rows 4z..4z+23 only (24x28 = 672 of 1008 cols):
    1.52x the true output volume instead of 2.29x dense.
  - Input DMAs ride the scalar + gpsimd queues (ordered by first use);
    dump DMAs ride the sync queue.  Tile deps are range-granular, so early
    blocks overlap the remaining input load.
  - Host extracts the valid (dy, dx) band with a single strided view
    (the per-partition "skew" is unexpressible by on-chip engines; numpy
    does it free).
"""

import sys

if "/opt/trn_rl_repo" not in sys.path:
    sys.path.insert(0, "/opt/trn_rl_repo")

import numpy as np

B, NIN, C, H, W = 4, 2, 128, 128, 128
KH = KW = 21
QS = np.float32(1e10)
HHALF = 64            # rows per core
HB, WB = 16, 8        # pixel block on stationary (M = 128)
NHB, NWB = HHALF // HB, W // WB      # 4, 16
RN2 = 18              # target rows per matmul (2 matmuls -> 36 = HB + 20)
WN = WB + 20          # 28 target cols per block
TROWS, TCOLS = HHALF + 20, W + 20    # 84, 148 padded target per core
SRC_F = HHALF * W                    # 8192
TGT_F = TROWS * TCOLS                # 12432
PACK_F = SRC_F + TGT_F
NBLK = NHB * NWB                     # 64
GRP = 8                              # blocks per staging group
NGRP = NBLK // GRP                   # 8
NZ = 4                               # partition zones per dump
ZP = 128 // NZ                       # 32 partitions per zone
ZROWS = HB // NZ + 20                # 24 t-rows per zone
ZCOLS = ZROWS * WN                   # 672
BLKF = 2 * RN2 * WN                  # 1008 staged cols per block

_nc_cache = None


def _build_nc():
    from contextlib import ExitStack

    from concourse import bacc, mybir, tile
    from concourse._compat import with_exitstack

    nc = bacc.Bacc("TRN2")
    dt_mm = mybir.dt.bfloat16
    dt_dump = mybir.dt.bfloat16
    pack = nc.declare_dram_parameter("pack", [C, PACK_F], dt_mm, isOutput=False)
    out = nc.declare_dram_parameter(
        "out", [NGRP, NZ, ZP, GRP, ZCOLS], dt_dump, isOutput=True
    )

    # input chunks, spread over the scalar + gpsimd queues, ordered by
    # first use (sync keeps the dump queue).
    def tgt_rng(t0, t1):
        return (SRC_F + t0 * TCOLS, SRC_F + t1 * TCOLS)

    def src_rng(b0, b1):
        return (b0 * 128, b1 * 128)

    scalar_chunks = [
        tgt_rng(0, 19), tgt_rng(19, 40), src_rng(16, 32), tgt_rng(61, TROWS)
    ]
    gpsimd_chunks = [
        src_rng(0, 4), src_rng(4, 16), tgt_rng(40, 61), src_rng(32, 48),
        src_rng(48, 64)
    ]

    @with_exitstack
    def kern(ctx: ExitStack, tc: tile.TileContext):
        nc = tc.nc
        sbp = ctx.enter_context(tc.tile_pool(name="inp", bufs=1))
        psa = ctx.enter_context(tc.tile_pool(name="psa", bufs=4, space="PSUM"))
        psb = ctx.enter_context(tc.tile_pool(name="psb", bufs=4, space="PSUM"))
        stp = ctx.enter_context(tc.tile_pool(name="stp", bufs=3))

        pk = sbp.tile([C, PACK_F], dt_mm, tag="pk")
        for lo, hi in scalar_chunks:
            nc.scalar.dma_start(pk[:, lo:hi], pack[:, lo:hi])
        for lo, hi in gpsimd_chunks:
            nc.gpsimd.dma_start(pk[:, lo:hi], pack[:, lo:hi])
        data = pk

        src2 = data[:, 0:SRC_F]
        tgt3 = data[:, SRC_F:].rearrange("c (t v) -> c t v", t=TROWS)

        for g in range(NGRP):
            sAB = stp.tile([128, GRP * BLKF], dt_dump)
            for k in range(GRP):
                blk = g * GRP + k
                hb, wb = divmod(blk, NWB)
                t0, w0 = hb * HB, wb * WB
                lhs = src2[:, blk * 128 : (blk + 1) * 128]
                pA = psa.tile([128, 504], mybir.dt.float32)
                pB = psb.tile([128, 504], mybir.dt.float32)
                nc.tensor.matmul(
                    pA[:], lhs, tgt3[:, t0 : t0 + RN2, w0 : w0 + WN],
                    start=True, stop=True,
                )
                nc.tensor.matmul(
                    pB[:], lhs, tgt3[:, t0 + RN2 : t0 + 2 * RN2, w0 : w0 + WN],
                    start=True, stop=True,
                )
                c0 = k * BLKF
                nc.vector.tensor_copy(sAB[:, c0 : c0 + 504], pA[:])
                nc.scalar.copy(sAB[:, c0 + 504 : c0 + BLKF], pB[:])
            for z in range(NZ):
                zsrc = (
                    sAB[z * ZP : (z + 1) * ZP, :]
                    .rearrange("p (k c) -> p k c", k=GRP)
                    [:, :, 4 * z * WN : 4 * z * WN + ZCOLS]
                )
                nc.sync.dma_start(out[g, z], zsrc)

    with tile.TileContext(nc) as tc:
        kern(tc)
    nc.finalize()
    return nc


def _get_nc():
    global _nc_cache
    if _nc_cache is None:
        _nc_cache = _build_nc()
    return _nc_cache


def _pack_inputs(q: np.ndarray) -> list[dict]:
    """Per-core packed input: blocked src + zero-padded tgt, bf16."""
    import ml_dtypes

    in_maps = []
    for core in range(8):
        b, half = core // 2, core % 2
        h0 = half * HHALF
        src = q[b, 0, :, h0 : h0 + HHALF, :]            # (C, 64, 128)
        srcb = (
            src.reshape(C, NHB, HB, NWB, WB)
            .transpose(0, 1, 3, 2, 4)                   # (C, hb, wb, h_l, w_l)
            .reshape(C, SRC_F)
        )
        tgt = np.zeros((C, TROWS, TCOLS), np.float32)
        lo, hi = h0 - 10, h0 + HHALF + 10
        clo, chi = max(lo, 0), min(hi, H)
        tgt[:, clo - lo : chi - lo, 10 : 10 + W] = q[b, 1, :, clo:chi, :]
        pack = np.concatenate([srcb, tgt.reshape(C, TGT_F)], axis=1)
        in_maps.append(
            {"pack": np.ascontiguousarray(pack).astype(ml_dtypes.bfloat16)}
        )
    return in_maps


def _unscramble(results: list[dict]) -> np.ndarray:
    """Extract the valid (dy, dx) band from each core's zoned Gram dump."""
    out = np.empty((B, KH * KW, H, W), np.float32)
    for core in range(8):
        b, half = core // 2, core % 2
        h0 = half * HHALF
        arr = np.asarray(results[core]["out"])
        if arr.dtype != np.float32:
            arr = arr.astype(np.float32)
        # [g, z, pp, k, col] with pp = hl4*8 + wl, col = (hl4 + dy)*28 + wl + dx
        arr = np.ascontiguousarray(arr.reshape(NGRP, NZ, ZP, GRP, ZCOLS))
        s_g, s_z, s_pp, s_k, s_c = arr.strides
        V = np.lib.stride_tricks.as_strided(
            arr,
            shape=(NHB, 2, GRP, NZ, 4, WB, KH, KW),
            # dims: hb, wbh, wbl, z, hl4, wl, dy, dx
            strides=(
                2 * s_g, s_g, s_k, s_z,
                8 * s_pp + WN * s_c, s_pp + s_c, WN * s_c, s_c,
            ),
        )
        # -> (dy, dx, hb, z, hl4, wbh, wbl, wl)
        oc = V.transpose(6, 7, 0, 3, 4, 1, 2, 5).reshape(KH * KW, HHALF, W)
        out[b, :, h0 : h0 + HHALF, :] = oc
    out *= np.float32(1.0 / C)
    return out


def _run(inputs: np.ndarray, trace: bool = False, trace_kwargs: dict | None = None):
    from concourse.bass_utils import run_bass_kernel_spmd

    x = np.asarray(inputs, dtype=np.float32)
    assert x.shape == (B, NIN, C, H, W), x.shape
    q = np.floor(x * QS) / QS        # fp32 ops, matches the jax reference
    in_maps = _pack_inputs(q)
    nc = _get_nc()
    res = run_bass_kernel_spmd(
        nc, in_maps, core_ids=list(range(8)), trace=trace,
        **(trace_kwargs or {}),
    )
    out = _unscramble(res.results)
    return out, res


def kernel(inputs: np.ndarray) -> np.ndarray:
    out, _ = _run(inputs, trace=False)
    return out


# revision 18
# speedup vs baseline: 1.0230x; 1.0230x over previous
"""Correlation-volume kernel for Trainium2 (8 NeuronCores, SPMD).

Problem: inputs (B=4, N=2, C=128, H=128, W=128) fp32.
  q = floor(inputs * 1e10) / 1e10  (straight-through quantization, fp32)
  src = q[:, 0], tgt = q[:, 1]
  out[b, dy*21+dx, h, w] = mean_c src[b,c,h,w] * tgt[b,c,h+dy-10,w+dx-10]
  (zero padding outside), out shape (4, 441, 128, 128) fp32.

Strategy:
  - Shard batch(4) x H-half(2) across 8 cores, data parallel, no collectives.
  - Host precomputes q, casts to bf16, pre-blocks src into 128-pixel
    stationary tiles (16 h x 8 w), zero-pads tgt; one packed input per core.
  - Device: per block, 2 bf16 matmuls (K=C=128, M=128 pixels,
    N=18 tgt rows x 28 tgt cols = 504) -> PSUM fp32; DVE/ACT copies cast to
    bf16 into a per-group staging tile.
  - Zoned dump: partition group h_l (8 partitions) only needs Gram t-rows
    h_l..h_l+20, so the 128 partitions split into 4 zones of 32 (h_l 4z..4z+3)
    and each zone dumps rows 4z..4z+23 only (24x28 = 672 of 1008 cols):
    1.52x the true output volume instead of 2.29x dense.
  - Input DMAs ride the scalar + gpsimd queues (ordered by first use);
    dump DMAs ride the sync queue.  Tile deps are range-granular, so early
    blocks overlap the remaining input load.
  - Host extracts the valid (dy, dx) band with a single strided view
    (the per-partition "skew" is unexpressible by on-chip engines; numpy
    does it free).
"""

import sys

if "/opt/trn_rl_repo" not in sys.path:
    sys.path.insert(0, "/opt/trn_rl_repo")

import numpy as np

B, NIN, C, H, W = 4, 2, 128, 128, 128
KH = KW = 21
QS = np.float32(1e10)
HHALF = 64            # rows per core
HB, WB = 16, 8        # pixel block on stationary (M = 128)
NHB, NWB = HHALF // HB, W // WB      # 4, 16
RN2 = 18              # target rows per matmul (2 matmuls -> 36 = HB + 20)
WN = WB + 20          # 28 target cols per block
TROWS, TCOLS = HHALF + 20, W + 20    # 84, 148 padded target per core
SRC_F = HHALF * W                    # 8192
TGT_F = TROWS * TCOLS                # 12432
PACK_F = SRC_F + TGT_F
NBLK = NHB * NWB                     # 64
GRP = 4                              # blocks per staging group
NGRP = NBLK // GRP                   # 16
NZ = 4                               # partition zones per dump
ZP = 128 // NZ                       # 32 partitions per zone
ZROWS = HB // NZ + 20                # 24 t-rows per zone
ZCOLS = ZROWS * WN                   # 672
BLKF = 2 * RN2 * WN                  # 1008 staged cols per block

_nc_cache = None


def _build_nc():
    from contextlib import ExitStack

    from concourse import bacc, mybir, tile
    from concourse._compat import with_exitstack

    nc = bacc.Bacc("TRN2")
    dt_mm = mybir.dt.bfloat16
    dt_dump = mybir.dt.bfloat16
    pack = nc.declare_dram_parameter("pack", [C, PACK_F], dt_mm, isOutput=False)
    out = nc.declare_dram_parameter(
        "out", [NGRP, NZ, ZP, GRP, ZCOLS], dt_dump, isOutput=True
    )

    # input chunks, spread over the scalar + gpsimd queues, ordered by
    # first use (sync keeps the dump queue).
    def tgt_rng(t0, t1):
        return (SRC_F + t0 * TCOLS, SRC_F + t1 * TCOLS)

    def src_rng(b0, b1):
        return (b0 * 128, b1 * 128)

    sync_chunks = [
        src_rng(0, 4), tgt_rng(0, 19), tgt_rng(19, 40), src_rng(4, 16),
        src_rng(16, 32), tgt_rng(40, 61), src_rng(32, 48), tgt_rng(61, TROWS),
        src_rng(48, 64),
    ]

    @with_exitstack
    def kern(ctx: ExitStack, tc: tile.TileContext):
        nc = tc.nc
        sbp = ctx.enter_context(tc.tile_pool(name="inp", bufs=1))
        psa = ctx.enter_context(tc.tile_pool(name="psa", bufs=4, space="PSUM"))
        psb = ctx.enter_context(tc.tile_pool(name="psb", bufs=4, space="PSUM"))
        stp = ctx.enter_context(tc.tile_pool(name="stp", bufs=3))

        pk = sbp.tile([C, PACK_F], dt_mm, tag="pk")
        for lo, hi in sync_chunks:
            nc.sync.dma_start(pk[:, lo:hi], pack[:, lo:hi])
        data = pk

        src2 = data[:, 0:SRC_F]
        tgt3 = data[:, SRC_F:].rearrange("c (t v) -> c t v", t=TROWS)

        for g in range(NGRP):
            sAB = stp.tile([128, GRP * BLKF], dt_dump)
            for k in range(GRP):
                blk = g * GRP + k
                hb, wb = divmod(blk, NWB)
                t0, w0 = hb * HB, wb * WB
                lhs = src2[:, blk * 128 : (blk + 1) * 128]
                pA = psa.tile([128, 504], mybir.dt.float32)
                pB = psb.tile([128, 504], mybir.dt.float32)
                nc.tensor.matmul(
                    pA[:], lhs, tgt3[:, t0 : t0 + RN2, w0 : w0 + WN],
                    start=True, stop=True,
                )
                nc.tensor.matmul(
                    pB[:], lhs, tgt3[:, t0 + RN2 : t0 + 2 * RN2, w0 : w0 + WN],
                    start=True, stop=True,
                )
                c0 = k * BLKF
                nc.vector.tensor_copy(sAB[:, c0 : c0 + 504], pA[:])
                nc.scalar.copy(sAB[:, c0 + 504 : c0 + BLKF], pB[:])
            for z in range(NZ):
                zsrc = (
                    sAB[z * ZP : (z + 1) * ZP, :]
                    .rearrange("p (k c) -> p k c", k=GRP)
                    [:, :, 4 * z * WN : 4 * z * WN + ZCOLS]
                )
                eng = nc.sync if z % 2 == 0 else nc.gpsimd
                eng.dma_start(out[g, z], zsrc)

    with tile.TileContext(nc) as tc:
        kern(tc)
    nc.finalize()
    return nc


def _get_nc():
    global _nc_cache
    if _nc_cache is None:
        _nc_cache = _build_nc()
    return _nc_cache


def _pack_inputs(q: np.ndarray) -> list[dict]:
    """Per-core packed input: blocked src + zero-padded tgt, bf16."""
    import ml_dtypes

    in_maps = []
    for core in range(8):
        b, half = core // 2, core % 2
        h0 = half * HHALF
        src = q[b, 0, :, h0 : h0 + HHALF, :]            # (C, 64, 128)
        srcb = (
            src.reshape(C, NHB, HB, NWB, WB)
            .transpose(0, 1, 3, 2, 4)                   # (C, hb, wb, h_l, w_l)
            .reshape(C, SRC_F)
        )
        tgt = np.zeros((C, TROWS, TCOLS), np.float32)
        lo, hi = h0 - 10, h0 + HHALF + 10
        clo, chi = max(lo, 0), min(hi, H)
        tgt[:, clo - lo : chi - lo, 10 : 10 + W] = q[b, 1, :, clo:chi, :]
        pack = np.concatenate([srcb, tgt.reshape(C, TGT_F)], axis=1)
        in_maps.append(
            {"pack": np.ascontiguousarray(pack).astype(ml_dtypes.bfloat16)}
        )
    return in_maps


def _unscramble(results: list[dict]) -> np.ndarray:
    """Extract the valid (dy, dx) band from each core's zoned Gram dump."""
    out = np.empty((B, KH * KW, H, W), np.float32)
    for core in range(8):
        b, half = core // 2, core % 2
        h0 = half * HHALF
        arr = np.asarray(results[core]["out"])
        if arr.dtype != np.float32:
            arr = arr.astype(np.float32)
        # [g, z, pp, k, col] with pp = hl4*8 + wl, col = (hl4 + dy)*28 + wl + dx
        arr = np.ascontiguousarray(arr.reshape(NGRP, NZ, ZP, GRP, ZCOLS))
        s_g, s_z, s_pp, s_k, s_c = arr.strides
        gpr = NWB // GRP             # groups per hb row
        V = np.lib.stride_tricks.as_strided(
            arr,
            shape=(NHB, gpr, GRP, NZ, 4, WB, KH, KW),
            # dims: hb, wbh, wbl, z, hl4, wl, dy, dx
            strides=(
                gpr * s_g, s_g, s_k, s_z,
                8 * s_pp + WN * s_c, s_pp + s_c, WN * s_c, s_c,
            ),
        )
        # -> (dy, dx, hb, z, hl4, wbh, wbl, wl)
        oc = V.transpose(6, 7, 0, 3, 4, 1, 2, 5).reshape(KH * KW, HHALF, W)
        out[b, :, h0 : h0 + HHALF, :] = oc
    out *= np.float32(1.0 / C)
    return out


def _run(inputs: np.ndarray, trace: bool = False, trace_kwargs: dict | None = None):
    from concourse.bass_utils import run_bass_kernel_spmd

    x = np.asarray(inputs, dtype=np.float32)
    assert x.shape == (B, NIN, C, H, W), x.shape
    q = np.floor(x * QS) / QS        # fp32 ops, matches the jax reference
    in_maps = _pack_inputs(q)
    nc = _get_nc()
    res = run_bass_kernel_spmd(
        nc, in_maps, core_ids=list(range(8)), trace=trace,
        **(trace_kwargs or {}),
    )
    out = _unscramble(res.results)
    return out, res


def kernel(inputs: np.ndarray) -> np.ndarray:
    out, _ = _run(inputs, trace=False)
    return out


# revision 20
# speedup vs baseline: 1.0970x; 1.0724x over previous
"""Correlation-volume kernel for Trainium2 (8 NeuronCores, SPMD).

Problem: inputs (B=4, N=2, C=128, H=128, W=128) fp32.
  q = floor(inputs * 1e10) / 1e10  (straight-through quantization, fp32)
  src = q[:, 0], tgt = q[:, 1]
  out[b, dy*21+dx, h, w] = mean_c src[b,c,h,w] * tgt[b,c,h+dy-10,w+dx-10]
  (zero padding outside), out shape (4, 441, 128, 128) fp32.

Strategy:
  - Shard batch(4) x H-half(2) across 8 cores, data parallel, no collectives.
  - Host precomputes q, casts to bf16, pre-blocks src into 128-pixel
    stationary tiles (16 h x 8 w), zero-pads tgt; one packed input per core.
  - Device: per block, 2 bf16 matmuls (K=C=128, M=128 pixels,
    N=18 tgt rows x 28 tgt cols = 504) -> PSUM fp32; DVE/ACT copies cast to
    bf16 into a per-group staging tile.
  - Zoned dump: partition group h_l (8 partitions) only needs Gram t-rows
    h_l..h_l+20, so the 128 partitions split into 4 zones of 32 (h_l 4z..4z+3)
    and each zone dumps rows 4z..4z+23 only (24x28 = 672 of 1008 cols):
    1.52x the true output volume instead of 2.29x dense.
  - Input DMAs ride the scalar + gpsimd queues (ordered by first use);
    dump DMAs ride the sync queue.  Tile deps are range-granular, so early
    blocks overlap the remaining input load.
  - Host extracts the valid (dy, dx) band with a single strided view
    (the per-partition "skew" is unexpressible by on-chip engines; numpy
    does it free).
"""

import sys

if "/opt/trn_rl_repo" not in sys.path:
    sys.path.insert(0, "/opt/trn_rl_repo")

import numpy as np

B, NIN, C, H, W = 4, 2, 128, 128, 128
KH = KW = 21
QS = np.float32(1e10)
HHALF = 64            # rows per core
HB, WB = 16, 8        # pixel block on stationary (M = 128)
NHB, NWB = HHALF // HB, W // WB      # 4, 16
RN2 = 18              # target rows per matmul (2 matmuls -> 36 = HB + 20)
WN = WB + 20          # 28 target cols per block
TROWS, TCOLS = HHALF + 20, W + 20    # 84, 148 padded target per core
SRC_F = HHALF * W                    # 8192
TGT_F = TROWS * TCOLS                # 12432
PACK_F = SRC_F + TGT_F
NBLK = NHB * NWB                     # 64
GRP = 4                              # blocks per staging group
NGRP = NBLK // GRP                   # 16
NZ = 4                               # partition zones per dump
ZP = 128 // NZ                       # 32 partitions per zone
ZROWS = HB // NZ + 20                # 24 t-rows per zone
ZCOLS = ZROWS * WN                   # 672
BLKF = 2 * RN2 * WN                  # 1008 staged cols per block

_nc_cache = None


def _build_nc():
    from contextlib import ExitStack

    from concourse import bacc, mybir, tile
    from concourse._compat import with_exitstack

    nc = bacc.Bacc("TRN2")
    dt_mm = mybir.dt.bfloat16
    dt_dump = mybir.dt.bfloat16
    pack = nc.declare_dram_parameter("pack", [C, PACK_F], dt_mm, isOutput=False)
    out = nc.declare_dram_parameter(
        "out", [NGRP, NZ, ZP, GRP, ZCOLS], dt_dump, isOutput=True
    )

    # input chunks, spread over the scalar + gpsimd queues, ordered by
    # first use (sync keeps the dump queue).
    def tgt_rng(t0, t1):
        return (SRC_F + t0 * TCOLS, SRC_F + t1 * TCOLS)

    def src_rng(b0, b1):
        return (b0 * 128, b1 * 128)

    sync_chunks = [
        src_rng(0, 4), tgt_rng(0, 19), tgt_rng(19, 40), src_rng(4, 16),
        src_rng(16, 32), tgt_rng(40, 61), src_rng(32, 48), tgt_rng(61, TROWS),
        src_rng(48, 64),
    ]

    @with_exitstack
    def kern(ctx: ExitStack, tc: tile.TileContext):
        nc = tc.nc
        sbp = ctx.enter_context(tc.tile_pool(name="inp", bufs=1))
        psp = ctx.enter_context(tc.tile_pool(name="psp", bufs=4, space="PSUM"))
        stp = ctx.enter_context(tc.tile_pool(name="stp", bufs=3))

        pk = sbp.tile([C, PACK_F], dt_mm, tag="pk")
        for lo, hi in sync_chunks:
            nc.sync.dma_start(pk[:, lo:hi], pack[:, lo:hi])
        data = pk

        src2 = data[:, 0:SRC_F]
        tgt3 = data[:, SRC_F:].rearrange("c (t v) -> c t v", t=TROWS)

        for g in range(NGRP):
            sAB = stp.tile([128, GRP * BLKF], dt_dump)
            for k in range(GRP):
                blk = g * GRP + k
                hb, wb = divmod(blk, NWB)
                t0, w0 = hb * HB, wb * WB
                lhs = src2[:, blk * 128 : (blk + 1) * 128]
                # one 2-bank PSUM tile per block (504-col matmuls at 512-col
                # bank-aligned offsets); freed by a single strided copy
                ps = psp.tile([128, 1024], mybir.dt.float32)
                nc.tensor.matmul(
                    ps[:, 0:504], lhs, tgt3[:, t0 : t0 + RN2, w0 : w0 + WN],
                    start=True, stop=True,
                )
                nc.tensor.matmul(
                    ps[:, 512:1016], lhs,
                    tgt3[:, t0 + RN2 : t0 + 2 * RN2, w0 : w0 + WN],
                    start=True, stop=True,
                )
                c0 = k * BLKF
                csrc = ps[:].rearrange("p (j c) -> p j c", j=2)[:, :, 0:504]
                cdst = sAB[:, c0 : c0 + BLKF].rearrange(
                    "p (j c) -> p j c", j=2
                )
                ceng = nc.vector if blk % 2 == 0 else nc.scalar
                if ceng is nc.vector:
                    ceng.tensor_copy(cdst, csrc)
                else:
                    ceng.copy(cdst, csrc)
            for z in range(NZ):
                zsrc = (
                    sAB[z * ZP : (z + 1) * ZP, :]
                    .rearrange("p (k c) -> p k c", k=GRP)
                    [:, :, 4 * z * WN : 4 * z * WN + ZCOLS]
                )
                eng = nc.sync if z % 2 == 0 else nc.gpsimd
                eng.dma_start(out[g, z], zsrc)

    with tile.TileContext(nc) as tc:
        kern(tc)
    nc.finalize()
    return nc


def _get_nc():
    global _nc_cache
    if _nc_cache is None:
        _nc_cache = _build_nc()
    return _nc_cache


def _pack_inputs(q: np.ndarray) -> list[dict]:
    """Per-core packed input: blocked src + zero-padded tgt, bf16."""
    import ml_dtypes

    in_maps = []
    for core in range(8):
        b, half = core // 2, core % 2
        h0 = half * HHALF
        src = q[b, 0, :, h0 : h0 + HHALF, :]            # (C, 64, 128)
        srcb = (
            src.reshape(C, NHB, HB, NWB, WB)
            .transpose(0, 1, 3, 2, 4)                   # (C, hb, wb, h_l, w_l)
            .reshape(C, SRC_F)
        )
        tgt = np.zeros((C, TROWS, TCOLS), np.float32)
        lo, hi = h0 - 10, h0 + HHALF + 10
        clo, chi = max(lo, 0), min(hi, H)
        tgt[:, clo - lo : chi - lo, 10 : 10 + W] = q[b, 1, :, clo:chi, :]
        pack = np.concatenate([srcb, tgt.reshape(C, TGT_F)], axis=1)
        in_maps.append(
            {"pack": np.ascontiguousarray(pack).astype(ml_dtypes.bfloat16)}
        )
    return in_maps


def _unscramble(results: list[dict]) -> np.ndarray:
    """Extract the valid (dy, dx) band from each core's zoned Gram dump."""
    out = np.empty((B, KH * KW, H, W), np.float32)
    for core in range(8):
        b, half = core // 2, core % 2
        h0 = half * HHALF
        arr = np.asarray(results[core]["out"])
        if arr.dtype != np.float32:
            arr = arr.astype(np.float32)
        # [g, z, pp, k, col] with pp = hl4*8 + wl, col = (hl4 + dy)*28 + wl + dx
        arr = np.ascontiguousarray(arr.reshape(NGRP, NZ, ZP, GRP, ZCOLS))
        s_g, s_z, s_pp, s_k, s_c = arr.strides
        gpr = NWB // GRP             # groups per hb row
        V = np.lib.stride_tricks.as_strided(
            arr,
            shape=(NHB, gpr, GRP, NZ, 4, WB, KH, KW),
            # dims: hb, wbh, wbl, z, hl4, wl, dy, dx
            strides=(
                gpr * s_g, s_g, s_k, s_z,
                8 * s_pp + WN * s_c, s_pp + s_c, WN * s_c, s_c,
            ),
        )
        # -> (dy, dx, hb, z, hl4, wbh, wbl, wl)
        oc = V.transpose(6, 7, 0, 3, 4, 1, 2, 5).reshape(KH * KW, HHALF, W)
        out[b, :, h0 : h0 + HHALF, :] = oc
    out *= np.float32(1.0 / C)
    return out


def _run(inputs: np.ndarray, trace: bool = False, trace_kwargs: dict | None = None):
    from concourse.bass_utils import run_bass_kernel_spmd

    x = np.asarray(inputs, dtype=np.float32)
    assert x.shape == (B, NIN, C, H, W), x.shape
    q = np.floor(x * QS) / QS        # fp32 ops, matches the jax reference
    in_maps = _pack_inputs(q)
    nc = _get_nc()
    res = run_bass_kernel_spmd(
        nc, in_maps, core_ids=list(range(8)), trace=trace,
        **(trace_kwargs or {}),
    )
    out = _unscramble(res.results)
    return out, res


def kernel(inputs: np.ndarray) -> np.ndarray:
    out, _ = _run(inputs, trace=False)
    return out


# revision 23
# speedup vs baseline: 1.1624x; 1.0597x over previous
"""Correlation-volume kernel for Trainium2 (8 NeuronCores, SPMD).

Problem: inputs (B=4, N=2, C=128, H=128, W=128) fp32.
  q = floor(inputs * 1e10) / 1e10  (straight-through quantization, fp32)
  src = q[:, 0], tgt = q[:, 1]
  out[b, dy*21+dx, h, w] = mean_c src[b,c,h,w] * tgt[b,c,h+dy-10,w+dx-10]
  (zero padding outside), out shape (4, 441, 128, 128) fp32.

Strategy:
  - Shard batch(4) x H-half(2) across 8 cores, data parallel, no collectives.
  - Host precomputes q, casts to bf16, pre-blocks src into 128-pixel
    stationary tiles (16 h x 8 w), zero-pads tgt; one packed input per core.
  - Device: per block, 2 bf16 matmuls (K=C=128, M=128 pixels,
    N=18 tgt rows x 28 tgt cols = 504) -> PSUM fp32; DVE/ACT copies cast to
    bf16 into a per-group staging tile.
  - Zoned dump: partition group h_l (8 partitions) only needs Gram t-rows
    h_l..h_l+20, so the 128 partitions split into 4 zones of 32 (h_l 4z..4z+3)
    and each zone dumps rows 4z..4z+23 only (24x28 = 672 of 1008 cols):
    1.52x the true output volume instead of 2.29x dense.
  - Input DMAs ride the scalar + gpsimd queues (ordered by first use);
    dump DMAs ride the sync queue.  Tile deps are range-granular, so early
    blocks overlap the remaining input load.
  - Host extracts the valid (dy, dx) band with a single strided view
    (the per-partition "skew" is unexpressible by on-chip engines; numpy
    does it free).
"""

import sys

if "/opt/trn_rl_repo" not in sys.path:
    sys.path.insert(0, "/opt/trn_rl_repo")

import numpy as np

B, NIN, C, H, W = 4, 2, 128, 128, 128
KH = KW = 21
QS = np.float32(1e10)
HHALF = 64            # rows per core
HB, WB = 16, 8        # pixel block on stationary (M = 128)
NHB, NWB = HHALF // HB, W // WB      # 4, 16
RN2 = 18              # target rows per matmul (2 matmuls -> 36 = HB + 20)
WN = WB + 20          # 28 target cols per block
TROWS, TCOLS = HHALF + 20, W + 20    # 84, 148 padded target per core
SRC_F = HHALF * W                    # 8192
TGT_F = TROWS * TCOLS                # 12432
PACK_F = SRC_F + TGT_F
NBLK = NHB * NWB                     # 64
GRP = 4                              # blocks per staging group
NGRP = NBLK // GRP                   # 16
NZ = 4                               # partition zones per dump
ZP = 128 // NZ                       # 32 partitions per zone
ZROWS = HB // NZ + 20                # 24 t-rows per zone
ZCOLS = ZROWS * WN                   # 672
BLKF = 2 * RN2 * WN                  # 1008 staged cols per block

_nc_cache = None


def _build_nc():
    from contextlib import ExitStack

    from concourse import bacc, mybir, tile
    from concourse._compat import with_exitstack

    nc = bacc.Bacc("TRN2")
    dt_mm = mybir.dt.bfloat16
    dt_dump = mybir.dt.bfloat16
    pack = nc.declare_dram_parameter("pack", [C, PACK_F], dt_mm, isOutput=False)
    out = nc.declare_dram_parameter(
        "out", [NGRP, NZ, ZP, GRP, ZCOLS], dt_dump, isOutput=True
    )

    # input chunks, spread over the scalar + gpsimd queues, ordered by
    # first use (sync keeps the dump queue).
    def tgt_rng(t0, t1):
        return (SRC_F + t0 * TCOLS, SRC_F + t1 * TCOLS)

    def src_rng(b0, b1):
        return (b0 * 128, b1 * 128)

    sync_chunks = [
        src_rng(0, 4), tgt_rng(19, 40), src_rng(16, 32), tgt_rng(61, TROWS),
        src_rng(48, 64),
    ]
    gp_chunks = [
        tgt_rng(0, 19), src_rng(4, 16), tgt_rng(40, 61), src_rng(32, 48),
    ]

    @with_exitstack
    def kern(ctx: ExitStack, tc: tile.TileContext):
        nc = tc.nc
        sbp = ctx.enter_context(tc.tile_pool(name="inp", bufs=1))
        psp = ctx.enter_context(tc.tile_pool(name="psp", bufs=4, space="PSUM"))
        stp = ctx.enter_context(tc.tile_pool(name="stp", bufs=4))

        pk = sbp.tile([C, PACK_F], dt_mm, tag="pk")
        for lo, hi in sync_chunks:
            nc.sync.dma_start(pk[:, lo:hi], pack[:, lo:hi])
        for lo, hi in gp_chunks:
            nc.gpsimd.dma_start(pk[:, lo:hi], pack[:, lo:hi])
        data = pk

        src2 = data[:, 0:SRC_F]
        tgt3 = data[:, SRC_F:].rearrange("c (t v) -> c t v", t=TROWS)

        for g in range(NGRP):
            sAB = stp.tile([128, GRP * BLKF], dt_dump)
            for k in range(GRP):
                blk = g * GRP + k
                hb, wb = divmod(blk, NWB)
                t0, w0 = hb * HB, wb * WB
                lhs = src2[:, blk * 128 : (blk + 1) * 128]
                # one 2-bank PSUM tile per block (504-col matmuls at 512-col
                # bank-aligned offsets); freed by a single strided copy
                ps = psp.tile([128, 1024], mybir.dt.float32)
                nc.tensor.matmul(
                    ps[:, 0:504], lhs, tgt3[:, t0 : t0 + RN2, w0 : w0 + WN],
                    start=True, stop=True,
                )
                nc.tensor.matmul(
                    ps[:, 512:1016], lhs,
                    tgt3[:, t0 + RN2 : t0 + 2 * RN2, w0 : w0 + WN],
                    start=True, stop=True,
                )
                c0 = k * BLKF
                csrc = ps[:].rearrange("p (j c) -> p j c", j=2)[:, :, 0:504]
                cdst = sAB[:, c0 : c0 + BLKF].rearrange(
                    "p (j c) -> p j c", j=2
                )
                ceng = nc.vector if blk % 2 == 0 else nc.scalar
                if ceng is nc.vector:
                    ceng.tensor_copy(cdst, csrc)
                else:
                    ceng.copy(cdst, csrc)
            for z in range(NZ):
                zsrc = (
                    sAB[z * ZP : (z + 1) * ZP, :]
                    .rearrange("p (k c) -> p k c", k=GRP)
                    [:, :, 4 * z * WN : 4 * z * WN + ZCOLS]
                )
                eng = nc.sync if z % 2 == 0 else nc.gpsimd
                eng.dma_start(out[g, z], zsrc)

    with tile.TileContext(nc) as tc:
        kern(tc)
    nc.finalize()
    return nc


def _get_nc():
    global _nc_cache
    if _nc_cache is None:
        _nc_cache = _build_nc()
    return _nc_cache


def _pack_inputs(q: np.ndarray) -> list[dict]:
    """Per-core packed input: blocked src + zero-padded tgt, bf16."""
    import ml_dtypes

    in_maps = []
    for core in range(8):
        b, half = core // 2, core % 2
        h0 = half * HHALF
        src = q[b, 0, :, h0 : h0 + HHALF, :]            # (C, 64, 128)
        srcb = (
            src.reshape(C, NHB, HB, NWB, WB)
            .transpose(0, 1, 3, 2, 4)                   # (C, hb, wb, h_l, w_l)
            .reshape(C, SRC_F)
        )
        tgt = np.zeros((C, TROWS, TCOLS), np.float32)
        lo, hi = h0 - 10, h0 + HHALF + 10
        clo, chi = max(lo, 0), min(hi, H)
        tgt[:, clo - lo : chi - lo, 10 : 10 + W] = q[b, 1, :, clo:chi, :]
        pack = np.concatenate([srcb, tgt.reshape(C, TGT_F)], axis=1)
        in_maps.append(
            {"pack": np.ascontiguousarray(pack).astype(ml_dtypes.bfloat16)}
        )
    return in_maps


def _unscramble(results: list[dict]) -> np.ndarray:
    """Extract the valid (dy, dx) band from each core's zoned Gram dump."""
    out = np.empty((B, KH * KW, H, W), np.float32)
    for core in range(8):
        b, half = core // 2, core % 2
        h0 = half * HHALF
        arr = np.asarray(results[core]["out"])
        if arr.dtype != np.float32:
            arr = arr.astype(np.float32)
        # [g, z, pp, k, col] with pp = hl4*8 + wl, col = (hl4 + dy)*28 + wl + dx
        arr = np.ascontiguousarray(arr.reshape(NGRP, NZ, ZP, GRP, ZCOLS))
        s_g, s_z, s_pp, s_k, s_c = arr.strides
        gpr = NWB // GRP             # groups per hb row
        V = np.lib.stride_tricks.as_strided(
            arr,
            shape=(NHB, gpr, GRP, NZ, 4, WB, KH, KW),
            # dims: hb, wbh, wbl, z, hl4, wl, dy, dx
            strides=(
                gpr * s_g, s_g, s_k, s_z,
                8 * s_pp + WN * s_c, s_pp + s_c, WN * s_c, s_c,
            ),
        )
        # -> (dy, dx, hb, z, hl4, wbh, wbl, wl)
        oc = V.transpose(6, 7, 0, 3, 4, 1, 2, 5).reshape(KH * KW, HHALF, W)
        out[b, :, h0 : h0 + HHALF, :] = oc
    out *= np.float32(1.0 / C)
    return out


def _run(inputs: np.ndarray, trace: bool = False, trace_kwargs: dict | None = None):
    from concourse.bass_utils import run_bass_kernel_spmd

    x = np.asarray(inputs, dtype=np.float32)
    assert x.shape == (B, NIN, C, H, W), x.shape
    q = np.floor(x * QS) / QS        # fp32 ops, matches the jax reference
    in_maps = _pack_inputs(q)
    nc = _get_nc()
    res = run_bass_kernel_spmd(
        nc, in_maps, core_ids=list(range(8)), trace=trace,
        **(trace_kwargs or {}),
    )
    out = _unscramble(res.results)
    return out, res


def kernel(inputs: np.ndarray) -> np.ndarray:
    out, _ = _run(inputs, trace=False)
    return out
